# revision 14
# baseline (speedup 1.0000x reference)
"""Causal single-head attention on 8 NeuronCores (Trainium2, Bass/Tile), v4.

Problem: x[16,4096,128] fp32; Wq/Wk/Wv[128,128]; y = softmax(mask(QK^T/sqrt(128))) @ V.
Sharding: data-parallel over batch, 2 batches per core, no collectives.
History: 394us naive -> 237us (v3) -> this.

v4 design (changes over v3):
  - ACT does NOTHING but exp: x/W are host-cast to fp16 (halves x DMA and
    kills the on-chip x cast), qT evacuation moved ACT->DVE, Vn fp8 cast
    moved DVE->GPSIMD. ACT budget ~= 144 exp calls ~= 139us.
  - phase A gets its OWN 2-bank PSUM pool (ps_a, 4 rotating 1-bank allocs
    per chunk: tp/q/k/V). v3 shared the pair pool, so every phase-A granule
    waited on a pair's exp to free PSUM -> ~1us PE stall every ~2.5us in
    the first 95us. Granules split finer (a1..a5) so each PE piece waits
    only on its own evacuation, absorbed by interleaved pair work.
  - x^T build: the x chunk DMA loads partition p <- rows {512c+4p+a}
    (2KB contiguous); transposing sub-tile a gives x^T columns l=4p+a at
    position 128a+32m+pp. The xt evacuation copy reorders to l-contiguous
    tile-major (dst-packed 4D AP; src strided breaks the DVE 2x mode but
    it's still ~0.7us vs v3's 1.3us un-interleave) so every matmul
    stationary AP stays 1-free-dim (a BIR verifier requirement) and the
    causal mask stays the plain triangular constant. The y store pays
    instead: partition p holds rows {512J+128t+p}, 4x512B runs.
  - causal mask add is a REGULAR f16 matmul (ident_h stationary, permuted
    mask as moving operand, accumulate into the diag strip) instead of
    v3's fp32 transpose-matmul: half the PE cost, same PE-only dep chain.
  - sums col-tiled: segments round-robin over 4 distinct 32-column PE
    array groups (ones[128,32] stationary, tile_position=(0,32g)) which
    run CONCURRENTLY -> ~4x less PE time than v3's full-width DR sums.
    Bands accumulate partials; epilogue contracts 4 bands with a 0/1
    selector column (the sel matmuls also transpose sums to per-partition
    layout for the reciprocal).
  - epilogue O^T->O transposes moved off the PE onto the DMA XBAR
    (dma_start_transpose, 4x [128,128] f16 per block on idle DMA HW).
  - late epilogue PSUM (sel-matmul output) borrows the dead tail of the
    block's diag2 pair slot (its strip is only 384 wide); late is deferred
    exactly ONE pair into the next block so the borrow's slot isn't yet
    reused and the smr copy has drained.
"""
import sys

if '/opt/trn_rl_repo' not in sys.path:
    sys.path.insert(0, '/opt/trn_rl_repo')

import numpy as np

B, L, D, H = 16, 4096, 128, 128
NCORES = 8
BPC = B // NCORES          # batches per core
QB = 512                   # q block width
NQB = L // QB              # 8 q blocks
KT = 128                   # k tile width
NKT = L // KT              # 32 k tiles
CHUNK = 512                # phase-A l-chunk
NCHUNK = L // CHUNK        # 8
SCALE = float(1.0 / np.sqrt(H))
NEG = -60000.0             # fp16-representable; SCALE*NEG << -80 so exp==0

_cache = {}


def _build():
    import concourse.mybir as mybir
    import concourse.tile as tile
    from concourse import bacc

    f32 = mybir.dt.float32
    f16 = mybir.dt.float16
    f8 = mybir.dt.float8e4
    DR = mybir.MatmulPerfMode.DoubleRow
    EXP = mybir.ActivationFunctionType.Exp

    nc = bacc.Bacc("TRN2", target_bir_lowering=False, debug=False,
                   num_devices=NCORES)
    x_ap = nc.dram_tensor("x16", [BPC, L, D], f16, kind="ExternalInput").ap()
    wq_ap = nc.dram_tensor("Wq16", [D, H], f16, kind="ExternalInput").ap()
    wk_ap = nc.dram_tensor("Wk16", [D, H], f16, kind="ExternalInput").ap()
    wv_ap = nc.dram_tensor("Wv16", [D, H], f16, kind="ExternalInput").ap()
    id_ap = nc.dram_tensor("ident16", [128, 128], f16, kind="ExternalInput").ap()
    mk_ap = nc.dram_tensor("mask16", [128, 128], f16, kind="ExternalInput").ap()
    sel_ap = nc.dram_tensor("sel16", [128, 1], f16, kind="ExternalInput").ap()
    y_ap = nc.dram_tensor("y", [BPC, L, H], f32, kind="ExternalOutput").ap()

    with tile.TileContext(nc) as tc:
        with (
            tc.tile_pool(name="const", bufs=1) as constp,
            tc.tile_pool(name="xchunk", bufs=4) as xchp,
            tc.tile_pool(name="xt", bufs=3) as xtp,
            tc.tile_pool(name="qkv", bufs=BPC) as qkvp,
            tc.tile_pool(name="pt", bufs=8) as ptp,
            tc.tile_pool(name="otsb", bufs=2) as otsbp,
            tc.tile_pool(name="ysp", bufs=2) as yspp,
            tc.tile_pool(name="smsb", bufs=2) as smsbp,
            tc.tile_pool(name="ysb", bufs=3) as yp,
            tc.tile_pool(name="ps_mm", bufs=2, space="PSUM") as ps_mm,
            tc.tile_pool(name="ps_a", bufs=2, space="PSUM") as ps_a,
            tc.tile_pool(name="ps_ot", bufs=1, space="PSUM") as ps_ot,
            tc.tile_pool(name="ps_sums", bufs=1, space="PSUM") as ps_sums,
        ):
            # ---- constants (all f16 direct from host; ACT stays exp-only) ----
            ident_h = constp.tile([128, 128], f16, tag="ident_h")
            nc.scalar.dma_start(ident_h[:], id_ap[:])
            mask_h = constp.tile([128, 128], f16, tag="mask_h")
            nc.scalar.dma_start(mask_h[:], mk_ap[:])
            sel_h = constp.tile([128, 1], f16, tag="sel_h")
            nc.scalar.dma_start(sel_h[:], sel_ap[:])
            w_h = {}
            for name, ap in (("q", wq_ap), ("k", wk_ap), ("v", wv_ap)):
                wh = constp.tile([128, 128], f16, tag=f"w{name}h")
                nc.scalar.dma_start(wh[:], ap[:])
                w_h[name] = wh
            ones_h = constp.tile([128, 32], f16, tag="ones_h")
            nc.gpsimd.memset(ones_h[:], 1.0)
            ones_8 = constp.tile([128, 32], f8, tag="ones_8")
            nc.gpsimd.memset(ones_8[:], 1.0)

            # ---- per-batch tensors ----
            qT = {}
            kT = {}
            Vn = {}
            Vh = {}
            xvs = {}
            yvs = {}
            for b in range(BPC):
                qT[b] = qkvp.tile([128, L], f16, tag="qT", name=f"qT{b}")
                kT[b] = qkvp.tile([128, L], f16, tag="kT", name=f"kT{b}")
                Vn[b] = qkvp.tile([128, L], f8, tag="V", name=f"V{b}")
                Vh[b] = qkvp.tile([128, L], f16, tag="Vh", name=f"Vh{b}")
                xvs[b] = x_ap[b].rearrange("(c p a) d -> c p (a d)", p=128, a=4)
                yvs[b] = y_ap[b].rearrange("(g t p) h -> g p t h", t=4, p=128)

            # ---- phase A granules (per chunk c of batch b) ----
            # x chunk DMA: partition p gets rows {512c+4p+a}, 2KB contiguous.
            # PSUM: 4 rotating 1-bank allocs from ps_a (tp, q, k, V); each
            # granule's PE part only ever waits on one of our own DVE
            # evacuations, never on a pair's exp.
            def phase_a1(b, c):
                xch = xchp.tile([128, 512], f16, tag="xch")
                nc.sync.dma_start(xch[:], xvs[b][c])
                tp_f32 = ps_a.tile([128, 512], f32, tag="a", name="tp")
                tp = tp_f32.bitcast(f16)[:, 0:512]
                for n in range(4):
                    nc.tensor.transpose(
                        tp[:, 128 * n:128 * (n + 1)],
                        xch[:, 128 * n:128 * (n + 1)], ident_h[:])
                return tp

            def phase_a2(b, c, tp):
                # reorder to l-contiguous: tp col 128a+32m+pp holds
                # l=4(32m+pp)+a; dst col 128m+4pp+a. dst last dim packed,
                # src strided (1 elem/cycle on DVE, ~0.7us).
                xt = xtp.tile([128, CHUNK], f16, tag="xt")
                nc.vector.tensor_copy(
                    xt[:].rearrange("d (m p a) -> d m p a", p=32, a=4),
                    tp.rearrange("d (a m p) -> d m p a", a=4, m=4))
                return xt

            def phase_a3(b, c, xt):
                pq = ps_a.tile([128, 512], f32, tag="a", name="pq")
                nc.tensor.matmul(pq[:], w_h["q"][:], xt[:],
                                 start=True, stop=True)
                nc.vector.tensor_copy(
                    qT[b][:, CHUNK * c:CHUNK * (c + 1)], pq[:])

            def phase_a4(b, c, xt):
                pk = ps_a.tile([128, 512], f32, tag="a", name="pk")
                nc.tensor.matmul(pk[:], w_h["k"][:], xt[:],
                                 start=True, stop=True)
                nc.vector.tensor_copy(
                    kT[b][:, CHUNK * c:CHUNK * (c + 1)], pk[:])

            def phase_a5(b, c, xt):
                # V tiles: per k-tile m of the chunk, lhsT = the 128 xt
                # columns with l in that tile = 2D AP [a:4][p:32m..32m+32];
                # V-tile partition s then holds k = 128i + sigma(s), the
                # same order the S^T stationary loads use.
                pv = ps_a.tile([128, 512], f32, tag="a", name="pv")
                for m in range(4):
                    nc.tensor.matmul(pv[:, 128 * m:128 * (m + 1)],
                                     xt[:, 128 * m:128 * (m + 1)],
                                     w_h["v"][:], start=True, stop=True)
                nc.vector.tensor_copy(
                    Vh[b][:, CHUNK * c:CHUNK * (c + 1)], pv[:])
                nc.gpsimd.tensor_copy(
                    Vn[b][:, CHUNK * c:CHUNK * (c + 1)],
                    Vh[b][:, CHUNK * c:CHUNK * (c + 1)])

            # ---- phase B block: pairs + col-tiled sums + split epilogue ----
            def phase_b(b, J, feed=None, pending_late=None):
                last_ret = {"late": None, "diag2": None}
                ot = ps_ot.tile([128, QB], f32, tag="ot")
                sm = ps_sums.tile([128, QB], f32, tag="sums")
                # Explicit zero: col-tiled sums accumulate with start=False
                # (a start=True clear is per-written-row-range; stale
                # has_written rows from the previous J would carry over).
                nc.vector.memset(sm[:], 0.0)

                pairs = []
                for g in range(2 * J):
                    pairs.append((2 * g, 2 * g + 1))
                pairs.append((4 * J, 4 * J + 1))
                pairs.append((4 * J + 2, 4 * J + 3))

                # sums: each k-tile segment is one [K=128, M=32, N] matmul
                # with ones stationary on col group (seg%4); the 4 matmuls
                # of a flush hit distinct 32-col PE groups and execute
                # concurrently. Bands hold partials; epilogue combines.
                state = {"seg": 0}
                pending = []
                nseg_total = 4 * J + 4

                def flush_sums():
                    for kind, src, qoff in pending:
                        g = state["seg"] % 4
                        last = state["seg"] == nseg_total - 1
                        ones = ones_8 if kind == "f8" else ones_h
                        nc.tensor.matmul(
                            sm[32 * g:32 * (g + 1), qoff:], ones[:], src,
                            start=False, stop=last,
                            skip_group_check=True,
                            tile_position=(0, 32 * g))
                        state["seg"] += 1
                    pending.clear()

                npv = 0
                npv_total = 2 * J + 4
                for pi, pair in enumerate(pairs):
                    stw = ps_mm.tile([128, 2 * QB], f32, tag="mm")
                    diag = pair[0] >= 4 * J
                    entries = []
                    cur = 0
                    for i in pair:
                        qoff = max(0, 128 * (i - 4 * J))
                        N = QB - qoff
                        nc.tensor.matmul(
                            stw[:, cur:cur + N],
                            kT[b][:, KT * i:KT * (i + 1)],
                            qT[b][:, QB * J + qoff:QB * (J + 1)],
                            start=True, stop=not diag,
                            skip_group_check=True)
                        if diag:
                            # causal mask on the triangle block: regular
                            # f16 matmul accumulate (ident stationary,
                            # mask moving) -- half the cost of v3's fp32
                            # transpose-matmul, same PE-only dep chain.
                            nc.tensor.matmul(
                                stw[:, cur:cur + 128],
                                ident_h[:], mask_h[:],
                                start=False, stop=True,
                                skip_group_check=True)
                        entries.append((i, qoff, cur, N))
                        cur += N
                    ptw = ptp.tile([128, 2 * QB], f16 if diag else f8,
                                   tag="pt" if diag else "pt8")
                    nc.scalar.activation(ptw[:, :cur], stw[:, :cur], EXP,
                                         scale=SCALE)
                    if not diag:
                        i0 = pair[0]
                        nc.tensor.matmul(
                            ot[:, 0:QB],
                            Vn[b][:, KT * i0:KT * (i0 + 2)].rearrange(
                                "p (o h) -> p o h", o=2),
                            ptw[:, 0:2 * QB].rearrange(
                                "p (o q) -> p o q", o=2),
                            start=(npv == 0), stop=(npv == npv_total - 1),
                            skip_group_check=True, perf_mode=DR)
                        npv += 1
                        pending.append(("f8", ptw[:, 0:QB], 0))
                        pending.append(("f8", ptw[:, QB:2 * QB], 0))
                    else:
                        for i, qoff, off, N in entries:
                            nc.tensor.matmul(ot[:, qoff:],
                                             Vh[b][:, KT * i:KT * (i + 1)],
                                             ptw[:, off:off + N],
                                             start=(npv == 0),
                                             stop=(npv == npv_total - 1),
                                             skip_group_check=True)
                            npv += 1
                            pending.append(("f16", ptw[:, off:off + N], qoff))
                    if len(pending) >= 4:
                        flush_sums()
                    if pi == 0 and pending_late is not None:
                        pending_late()
                    if feed is not None:
                        feed()
                    if diag and pair[0] == 4 * J + 2:
                        last_ret["diag2"] = stw
                flush_sums()

                # early epilogue (DVE only): evacuate ot and sm promptly
                otsb = otsbp.tile([128, QB], f16, tag="otsb")
                nc.vector.tensor_copy(otsb[:], ot[:])
                smr = smsbp.tile([128, QB], f16, tag="smsb")
                nc.vector.tensor_copy(smr[:], sm[:])
                diag2 = last_ret["diag2"]

                def late():
                    # sel matmuls: combine the 4 col-group bands AND
                    # transpose sums into per-partition layout.
                    # stp borrows the dead tail of this block's diag2 pair
                    # slot (its strip is only 384 wide).
                    stp = diag2[:, 384:512]
                    for a in range(4):
                        # independent 1-col groups: start=True each (a
                        # shared group would accumulate onto stale
                        # has_written state in the borrowed region)
                        nc.tensor.matmul(stp[:, a:a + 1],
                                         smr[:, 128 * a:128 * (a + 1)],
                                         sel_h[:],
                                         start=True, stop=True,
                                         skip_group_check=True)
                    rcp = smsbp.tile([128, 4], f32, tag="rcp")
                    nc.vector.reciprocal(rcp[:], stp[:, 0:4])
                    # O^T -> O on the DMA XBAR (sub-tile a holds q=4p+a,
                    # matching the y store interleave), normalize on DVE.
                    ysp = yspp.tile([128, QB], f16, tag="ysp")
                    ys = yp.tile([128, QB], f32, tag="y")
                    for a in range(4):
                        nc.sync.dma_start_transpose(
                            ysp[:, 128 * a:128 * (a + 1)],
                            otsb[:, 128 * a:128 * (a + 1)])
                    for a in range(4):
                        nc.vector.tensor_scalar_mul(
                            ys[:, 128 * a:128 * (a + 1)],
                            ysp[:, 128 * a:128 * (a + 1)], rcp[:, a:a + 1])
                    nc.sync.dma_start(
                        yvs[b][J], ys[:].rearrange("p (t h) -> p t h", t=4))

                return late

            # ---- interleaved emission schedule ----
            gran = []
            for b in range(BPC):
                for c in range(NCHUNK):
                    gran.append((b, c))
            gstate = {"idx": 0, "sub": 0, "tp": None, "xt": None,
                      "pairs": 0}
            NSUB = 5

            def emit_sub():
                b, c = gran[gstate["idx"]]
                s = gstate["sub"]
                if s == 0:
                    gstate["tp"] = phase_a1(b, c)
                elif s == 1:
                    gstate["xt"] = phase_a2(b, c, gstate["tp"])
                elif s == 2:
                    phase_a3(b, c, gstate["xt"])
                elif s == 3:
                    phase_a4(b, c, gstate["xt"])
                else:
                    phase_a5(b, c, gstate["xt"])
                gstate["sub"] += 1
                if gstate["sub"] == NSUB:
                    gstate["sub"] = 0
                    gstate["idx"] += 1

            def gpos():
                return gstate["idx"] * NSUB + gstate["sub"]

            NGRAN = len(gran) * NSUB
            NPAIRS = sum(2 * J + 2 for J in range(NQB)) * BPC

            def feed():
                gstate["pairs"] += 1
                target = min(NGRAN, 8 + gstate["pairs"] * NGRAN // NPAIRS)
                while gpos() < target:
                    emit_sub()

            border = [(0, J) for J in range(4)]
            for J in range(4):
                border += [(0, 4 + J), (1, J)]
            border += [(1, J) for J in range(4, NQB)]

            pending_late = None
            for b, J in border:
                while gpos() < ((b * NCHUNK + J) + 1) * NSUB:
                    emit_sub()
                late = phase_b(b, J, feed=feed, pending_late=pending_late)
                pending_late = late
            if pending_late is not None:
                pending_late()
    nc.compile()
    return nc


def _host_consts():
    ident = np.eye(128, dtype=np.float16)
    # rows s (S^T k-row), cols j (q position within the triangle block):
    # keep iff q >= k <=> j >= s
    t = np.arange(128)
    mask = np.where(t[None, :] >= t[:, None], 0.0, NEG).astype(np.float16)
    sel = np.zeros((128, 1), dtype=np.float16)
    sel[0::32, 0] = 1.0
    return ident, mask, sel


def kernel(x, Wq, Wk, Wv):
    from concourse import bass_utils

    if "nc" not in _cache:
        _cache["nc"] = _build()
    nc = _cache["nc"]

    x16 = np.ascontiguousarray(x, dtype=np.float16)
    ident, mask, sel = _host_consts()
    in_maps = []
    for c in range(NCORES):
        in_maps.append({
            "x16": x16[BPC * c:BPC * (c + 1)],
            "Wq16": np.ascontiguousarray(Wq, dtype=np.float16),
            "Wk16": np.ascontiguousarray(Wk, dtype=np.float16),
            "Wv16": np.ascontiguousarray(Wv, dtype=np.float16),
            "ident16": ident,
            "mask16": mask,
            "sel16": sel,
        })
    res = bass_utils.run_bass_kernel_spmd(nc, in_maps,
                                          core_ids=list(range(NCORES)))
    _cache["last_results"] = res
    y = np.concatenate([res.results[c]["y"] for c in range(NCORES)], axis=0)
    return y


# revision 16
# speedup vs baseline: 1.0527x; 1.0527x over previous
"""Causal single-head attention on 8 NeuronCores (Trainium2, Bass/Tile), v4.

Problem: x[16,4096,128] fp32; Wq/Wk/Wv[128,128]; y = softmax(mask(QK^T/sqrt(128))) @ V.
Sharding: data-parallel over batch, 2 batches per core, no collectives.
History: 394us naive -> 237us (v3) -> this.

v4 design (changes over v3):
  - ACT does NOTHING but exp: x/W are host-cast to fp16 (halves x DMA and
    kills the on-chip x cast), qT evacuation moved ACT->DVE, Vn fp8 cast
    moved DVE->GPSIMD. ACT budget ~= 144 exp calls ~= 139us.
  - phase A gets its OWN 2-bank PSUM pool (ps_a, 4 rotating 1-bank allocs
    per chunk: tp/q/k/V). v3 shared the pair pool, so every phase-A granule
    waited on a pair's exp to free PSUM -> ~1us PE stall every ~2.5us in
    the first 95us. Granules split finer (a1..a5) so each PE piece waits
    only on its own evacuation, absorbed by interleaved pair work.
  - x^T build: the x chunk DMA loads partition p <- rows {512c+4p+a}
    (2KB contiguous); transposing sub-tile a gives x^T columns l=4p+a at
    position 128a+32m+pp. The xt evacuation copy reorders to l-contiguous
    tile-major (dst-packed 4D AP; src strided breaks the DVE 2x mode but
    it's still ~0.7us vs v3's 1.3us un-interleave) so every matmul
    stationary AP stays 1-free-dim (a BIR verifier requirement) and the
    causal mask stays the plain triangular constant. The y store pays
    instead: partition p holds rows {512J+128t+p}, 4x512B runs.
  - causal mask add is a REGULAR f16 matmul (ident_h stationary, permuted
    mask as moving operand, accumulate into the diag strip) instead of
    v3's fp32 transpose-matmul: half the PE cost, same PE-only dep chain.
  - sums col-tiled: segments round-robin over 4 distinct 32-column PE
    array groups (ones[128,32] stationary, tile_position=(0,32g)) which
    run CONCURRENTLY -> ~4x less PE time than v3's full-width DR sums.
    Bands accumulate partials; epilogue contracts 4 bands with a 0/1
    selector column (the sel matmuls also transpose sums to per-partition
    layout for the reciprocal).
  - epilogue O^T->O transposes moved off the PE onto the DMA XBAR
    (dma_start_transpose, 4x [128,128] f16 per block on idle DMA HW).
  - late epilogue PSUM (sel-matmul output) borrows the dead tail of the
    block's diag2 pair slot (its strip is only 384 wide); late is deferred
    exactly ONE pair into the next block so the borrow's slot isn't yet
    reused and the smr copy has drained.
"""
import sys

if '/opt/trn_rl_repo' not in sys.path:
    sys.path.insert(0, '/opt/trn_rl_repo')

import numpy as np

B, L, D, H = 16, 4096, 128, 128
NCORES = 8
BPC = B // NCORES          # batches per core
QB = 512                   # q block width
NQB = L // QB              # 8 q blocks
KT = 128                   # k tile width
NKT = L // KT              # 32 k tiles
CHUNK = 512                # phase-A l-chunk
NCHUNK = L // CHUNK        # 8
SCALE = float(1.0 / np.sqrt(H))
NEG = -60000.0             # fp16-representable; SCALE*NEG << -80 so exp==0

_cache = {}


def _build():
    import concourse.mybir as mybir
    import concourse.tile as tile
    from concourse import bacc

    f32 = mybir.dt.float32
    f16 = mybir.dt.float16
    f8 = mybir.dt.float8e4
    DR = mybir.MatmulPerfMode.DoubleRow
    EXP = mybir.ActivationFunctionType.Exp

    nc = bacc.Bacc("TRN2", target_bir_lowering=False, debug=False,
                   num_devices=NCORES)
    x_ap = nc.dram_tensor("x16", [BPC, L, D], f16, kind="ExternalInput").ap()
    wq_ap = nc.dram_tensor("Wq16", [D, H], f16, kind="ExternalInput").ap()
    wk_ap = nc.dram_tensor("Wk16", [D, H], f16, kind="ExternalInput").ap()
    wv_ap = nc.dram_tensor("Wv16", [D, H], f16, kind="ExternalInput").ap()
    id_ap = nc.dram_tensor("ident16", [128, 128], f16, kind="ExternalInput").ap()
    mk_ap = nc.dram_tensor("mask16", [128, 128], f16, kind="ExternalInput").ap()
    sel_ap = nc.dram_tensor("sel16", [128, 1], f16, kind="ExternalInput").ap()
    y_ap = nc.dram_tensor("y", [BPC, L, H], f32, kind="ExternalOutput").ap()

    with tile.TileContext(nc) as tc:
        with (
            tc.tile_pool(name="const", bufs=1) as constp,
            tc.tile_pool(name="xchunk", bufs=4) as xchp,
            tc.tile_pool(name="xt", bufs=3) as xtp,
            tc.tile_pool(name="qkv", bufs=BPC) as qkvp,
            tc.tile_pool(name="pt", bufs=8) as ptp,
            tc.tile_pool(name="otsb", bufs=2) as otsbp,
            tc.tile_pool(name="smsb", bufs=2) as smsbp,
            tc.tile_pool(name="ysb", bufs=3) as yp,
            tc.tile_pool(name="ps_mm", bufs=2, space="PSUM") as ps_mm,
            tc.tile_pool(name="ps_a", bufs=2, space="PSUM") as ps_a,
            tc.tile_pool(name="ps_ot", bufs=1, space="PSUM") as ps_ot,
            tc.tile_pool(name="ps_sums", bufs=1, space="PSUM") as ps_sums,
        ):
            # ---- constants (all f16 direct from host; ACT stays exp-only) ----
            ident_h = constp.tile([128, 128], f16, tag="ident_h")
            nc.scalar.dma_start(ident_h[:], id_ap[:])
            mask_h = constp.tile([128, 128], f16, tag="mask_h")
            nc.scalar.dma_start(mask_h[:], mk_ap[:])
            sel_h = constp.tile([128, 1], f16, tag="sel_h")
            nc.scalar.dma_start(sel_h[:], sel_ap[:])
            w_h = {}
            for name, ap in (("q", wq_ap), ("k", wk_ap), ("v", wv_ap)):
                wh = constp.tile([128, 128], f16, tag=f"w{name}h")
                nc.scalar.dma_start(wh[:], ap[:])
                w_h[name] = wh
            ones_h = constp.tile([128, 32], f16, tag="ones_h")
            nc.gpsimd.memset(ones_h[:], 1.0)
            ones_8 = constp.tile([128, 32], f8, tag="ones_8")
            nc.gpsimd.memset(ones_8[:], 1.0)

            # ---- per-batch tensors ----
            qT = {}
            kT = {}
            Vn = {}
            Vh = {}
            xvs = {}
            yvs = {}
            for b in range(BPC):
                qT[b] = qkvp.tile([128, L], f16, tag="qT", name=f"qT{b}")
                kT[b] = qkvp.tile([128, L], f16, tag="kT", name=f"kT{b}")
                Vn[b] = qkvp.tile([128, L], f8, tag="V", name=f"V{b}")
                Vh[b] = qkvp.tile([128, L], f16, tag="Vh", name=f"Vh{b}")
                xvs[b] = x_ap[b].rearrange("(c p a) d -> c p (a d)", p=128, a=4)
                yvs[b] = y_ap[b].rearrange("(g t p) h -> g p t h", t=4, p=128)

            # ---- phase A granules (per chunk c of batch b) ----
            # x chunk DMA: partition p gets rows {512c+4p+a}, 2KB contiguous.
            # PSUM: 4 rotating 1-bank allocs from ps_a (tp, q, k, V); each
            # granule's PE part only ever waits on one of our own DVE
            # evacuations, never on a pair's exp.
            def phase_a1(b, c):
                xch = xchp.tile([128, 512], f16, tag="xch")
                nc.sync.dma_start(xch[:], xvs[b][c])
                tp_f32 = ps_a.tile([128, 512], f32, tag="a", name="tp")
                tp = tp_f32.bitcast(f16)[:, 0:512]
                for n in range(4):
                    nc.tensor.transpose(
                        tp[:, 128 * n:128 * (n + 1)],
                        xch[:, 128 * n:128 * (n + 1)], ident_h[:])
                return tp

            def phase_a2(b, c, tp):
                # reorder to l-contiguous: tp col 128a+32m+pp holds
                # l=4(32m+pp)+a; dst col 128m+4pp+a. dst last dim packed,
                # src strided (1 elem/cycle on DVE, ~0.7us).
                xt = xtp.tile([128, CHUNK], f16, tag="xt")
                nc.vector.tensor_copy(
                    xt[:].rearrange("d (m p a) -> d m p a", p=32, a=4),
                    tp.rearrange("d (a m p) -> d m p a", a=4, m=4))
                return xt

            def phase_a3(b, c, xt):
                pq = ps_a.tile([128, 512], f32, tag="a", name="pq")
                nc.tensor.matmul(pq[:], w_h["q"][:], xt[:],
                                 start=True, stop=True)
                nc.vector.tensor_copy(
                    qT[b][:, CHUNK * c:CHUNK * (c + 1)], pq[:])

            def phase_a4(b, c, xt):
                pk = ps_a.tile([128, 512], f32, tag="a", name="pk")
                nc.tensor.matmul(pk[:], w_h["k"][:], xt[:],
                                 start=True, stop=True)
                nc.vector.tensor_copy(
                    kT[b][:, CHUNK * c:CHUNK * (c + 1)], pk[:])

            def phase_a5(b, c, xt):
                # V tiles: per k-tile m of the chunk, lhsT = the 128 xt
                # columns with l in that tile = 2D AP [a:4][p:32m..32m+32];
                # V-tile partition s then holds k = 128i + sigma(s), the
                # same order the S^T stationary loads use.
                pv = ps_a.tile([128, 512], f32, tag="a", name="pv")
                for m in range(4):
                    nc.tensor.matmul(pv[:, 128 * m:128 * (m + 1)],
                                     xt[:, 128 * m:128 * (m + 1)],
                                     w_h["v"][:], start=True, stop=True)
                nc.vector.tensor_copy(
                    Vh[b][:, CHUNK * c:CHUNK * (c + 1)], pv[:])
                nc.gpsimd.tensor_copy(
                    Vn[b][:, CHUNK * c:CHUNK * (c + 1)],
                    Vh[b][:, CHUNK * c:CHUNK * (c + 1)])

            # ---- phase B block: pairs + col-tiled sums + split epilogue ----
            def phase_b(b, J, feed=None, pending_late=None):
                last_ret = {"late": None, "diag2": None}
                ot = ps_ot.tile([128, QB], f32, tag="ot")
                sm = ps_sums.tile([128, QB], f32, tag="sums")
                # Explicit zero: col-tiled sums accumulate with start=False
                # (a start=True clear is per-written-row-range; stale
                # has_written rows from the previous J would carry over).
                nc.vector.memset(sm[:], 0.0)

                pairs = []
                for g in range(2 * J):
                    pairs.append((2 * g, 2 * g + 1))
                pairs.append((4 * J, 4 * J + 1))
                pairs.append((4 * J + 2, 4 * J + 3))

                # sums: each k-tile segment is one [K=128, M=32, N] matmul
                # with ones stationary on col group (seg%4); the 4 matmuls
                # of a flush hit distinct 32-col PE groups and execute
                # concurrently. Bands hold partials; epilogue combines.
                state = {"seg": 0}
                pending = []
                nseg_total = 4 * J + 4

                def flush_sums():
                    for kind, src, qoff in pending:
                        g = state["seg"] % 4
                        last = state["seg"] == nseg_total - 1
                        ones = ones_8 if kind == "f8" else ones_h
                        nc.tensor.matmul(
                            sm[32 * g:32 * (g + 1), qoff:], ones[:], src,
                            start=False, stop=last,
                            skip_group_check=True,
                            tile_position=(0, 32 * g))
                        state["seg"] += 1
                    pending.clear()

                npv = 0
                npv_total = 2 * J + 4
                for pi, pair in enumerate(pairs):
                    stw = ps_mm.tile([128, 2 * QB], f32, tag="mm")
                    diag = pair[0] >= 4 * J
                    entries = []
                    cur = 0
                    for i in pair:
                        qoff = max(0, 128 * (i - 4 * J))
                        N = QB - qoff
                        nc.tensor.matmul(
                            stw[:, cur:cur + N],
                            kT[b][:, KT * i:KT * (i + 1)],
                            qT[b][:, QB * J + qoff:QB * (J + 1)],
                            start=True, stop=not diag,
                            skip_group_check=True)
                        if diag:
                            # causal mask on the triangle block: regular
                            # f16 matmul accumulate (ident stationary,
                            # mask moving) -- half the cost of v3's fp32
                            # transpose-matmul, same PE-only dep chain.
                            nc.tensor.matmul(
                                stw[:, cur:cur + 128],
                                ident_h[:], mask_h[:],
                                start=False, stop=True,
                                skip_group_check=True)
                        entries.append((i, qoff, cur, N))
                        cur += N
                    ptw = ptp.tile([128, 2 * QB], f16 if diag else f8,
                                   tag="pt" if diag else "pt8")
                    nc.scalar.activation(ptw[:, :cur], stw[:, :cur], EXP,
                                         scale=SCALE)
                    if not diag:
                        i0 = pair[0]
                        nc.tensor.matmul(
                            ot[:, 0:QB],
                            Vn[b][:, KT * i0:KT * (i0 + 2)].rearrange(
                                "p (o h) -> p o h", o=2),
                            ptw[:, 0:2 * QB].rearrange(
                                "p (o q) -> p o q", o=2),
                            start=(npv == 0), stop=(npv == npv_total - 1),
                            skip_group_check=True, perf_mode=DR)
                        npv += 1
                        pending.append(("f8", ptw[:, 0:QB], 0))
                        pending.append(("f8", ptw[:, QB:2 * QB], 0))
                    else:
                        for i, qoff, off, N in entries:
                            nc.tensor.matmul(ot[:, qoff:],
                                             Vh[b][:, KT * i:KT * (i + 1)],
                                             ptw[:, off:off + N],
                                             start=(npv == 0),
                                             stop=(npv == npv_total - 1),
                                             skip_group_check=True)
                            npv += 1
                            pending.append(("f16", ptw[:, off:off + N], qoff))
                    if len(pending) >= 4:
                        flush_sums()
                    if pi == 0 and pending_late is not None:
                        pending_late()
                    if feed is not None:
                        feed()
                    if diag and pair[0] == 4 * J + 2:
                        last_ret["diag2"] = stw
                flush_sums()

                # early epilogue (DVE only): evacuate ot and sm promptly
                otsb = otsbp.tile([128, QB], f16, tag="otsb")
                nc.vector.tensor_copy(otsb[:], ot[:])
                smr = smsbp.tile([128, QB], f16, tag="smsb")
                nc.vector.tensor_copy(smr[:], sm[:])
                diag2 = last_ret["diag2"]

                def late():
                    # sel matmuls: combine the 4 col-group bands AND
                    # transpose sums into per-partition layout.
                    # stp borrows the dead tail of this block's diag2 pair
                    # slot (its strip is only 384 wide).
                    stp = diag2[:, 384:512]
                    for a in range(4):
                        # independent 1-col groups: start=True each (a
                        # shared group would accumulate onto stale
                        # has_written state in the borrowed region)
                        nc.tensor.matmul(stp[:, a:a + 1],
                                         smr[:, 128 * a:128 * (a + 1)],
                                         sel_h[:],
                                         start=True, stop=True,
                                         skip_group_check=True)
                    rcp = smsbp.tile([128, 4], f32, tag="rcp")
                    nc.vector.reciprocal(rcp[:], stp[:, 0:4])
                    # O^T -> O: f16 PE transposes into another dead piece
                    # of the diag2 slot (f32 cols [512:768) as f16), then
                    # normalize on DVE. (DMA XBAR transposes were tried:
                    # 1.25us serialized triggers on the sync queue gated
                    # the DVE queue and starved the PE.)
                    op = diag2.bitcast(f16)[:, 1024:1536]
                    ys = yp.tile([128, QB], f32, tag="y")
                    for a in range(4):
                        nc.tensor.transpose(
                            op[:, 128 * a:128 * (a + 1)],
                            otsb[:, 128 * a:128 * (a + 1)], ident_h[:])
                    for a in range(4):
                        nc.vector.tensor_scalar_mul(
                            ys[:, 128 * a:128 * (a + 1)],
                            op[:, 128 * a:128 * (a + 1)], rcp[:, a:a + 1])
                    nc.sync.dma_start(
                        yvs[b][J], ys[:].rearrange("p (t h) -> p t h", t=4))

                return late

            # ---- interleaved emission schedule ----
            gran = []
            for b in range(BPC):
                for c in range(NCHUNK):
                    gran.append((b, c))
            gstate = {"idx": 0, "sub": 0, "tp": None, "xt": None,
                      "pairs": 0}
            NSUB = 5

            def emit_sub():
                b, c = gran[gstate["idx"]]
                s = gstate["sub"]
                if s == 0:
                    gstate["tp"] = phase_a1(b, c)
                elif s == 1:
                    gstate["xt"] = phase_a2(b, c, gstate["tp"])
                elif s == 2:
                    phase_a3(b, c, gstate["xt"])
                elif s == 3:
                    phase_a4(b, c, gstate["xt"])
                else:
                    phase_a5(b, c, gstate["xt"])
                gstate["sub"] += 1
                if gstate["sub"] == NSUB:
                    gstate["sub"] = 0
                    gstate["idx"] += 1

            def gpos():
                return gstate["idx"] * NSUB + gstate["sub"]

            NGRAN = len(gran) * NSUB
            NPAIRS = sum(2 * J + 2 for J in range(NQB)) * BPC

            def feed():
                gstate["pairs"] += 1
                target = min(NGRAN, 8 + gstate["pairs"] * NGRAN // NPAIRS)
                while gpos() < target:
                    emit_sub()

            border = [(0, J) for J in range(4)]
            for J in range(4):
                border += [(0, 4 + J), (1, J)]
            border += [(1, J) for J in range(4, NQB)]

            pending_late = None
            for b, J in border:
                while gpos() < ((b * NCHUNK + J) + 1) * NSUB:
                    emit_sub()
                late = phase_b(b, J, feed=feed, pending_late=pending_late)
                pending_late = late
            if pending_late is not None:
                pending_late()
    nc.compile()
    return nc


def _host_consts():
    ident = np.eye(128, dtype=np.float16)
    # rows s (S^T k-row), cols j (q position within the triangle block):
    # keep iff q >= k <=> j >= s
    t = np.arange(128)
    mask = np.where(t[None, :] >= t[:, None], 0.0, NEG).astype(np.float16)
    sel = np.zeros((128, 1), dtype=np.float16)
    sel[0::32, 0] = 1.0
    return ident, mask, sel


def kernel(x, Wq, Wk, Wv):
    from concourse import bass_utils

    if "nc" not in _cache:
        _cache["nc"] = _build()
    nc = _cache["nc"]

    x16 = np.ascontiguousarray(x, dtype=np.float16)
    ident, mask, sel = _host_consts()
    in_maps = []
    for c in range(NCORES):
        in_maps.append({
            "x16": x16[BPC * c:BPC * (c + 1)],
            "Wq16": np.ascontiguousarray(Wq, dtype=np.float16),
            "Wk16": np.ascontiguousarray(Wk, dtype=np.float16),
            "Wv16": np.ascontiguousarray(Wv, dtype=np.float16),
            "ident16": ident,
            "mask16": mask,
            "sel16": sel,
        })
    res = bass_utils.run_bass_kernel_spmd(nc, in_maps,
                                          core_ids=list(range(NCORES)))
    _cache["last_results"] = res
    y = np.concatenate([res.results[c]["y"] for c in range(NCORES)], axis=0)
    return y


# revision 18
# speedup vs baseline: 1.1776x; 1.1187x over previous
"""Causal single-head attention on 8 NeuronCores (Trainium2, Bass/Tile), v4.

Problem: x[16,4096,128] fp32; Wq/Wk/Wv[128,128]; y = softmax(mask(QK^T/sqrt(128))) @ V.
Sharding: data-parallel over batch, 2 batches per core, no collectives.
History: 394us naive -> 237us (v3) -> this.

v4 design (changes over v3):
  - ACT does NOTHING but exp: x/W are host-cast to fp16 (halves x DMA and
    kills the on-chip x cast), qT evacuation moved ACT->DVE, Vn fp8 cast
    moved DVE->GPSIMD. ACT budget ~= 144 exp calls ~= 139us.
  - phase A gets its OWN 2-bank PSUM pool (ps_a, 4 rotating 1-bank allocs
    per chunk: tp/q/k/V). v3 shared the pair pool, so every phase-A granule
    waited on a pair's exp to free PSUM -> ~1us PE stall every ~2.5us in
    the first 95us. Granules split finer (a1..a5) so each PE piece waits
    only on its own evacuation, absorbed by interleaved pair work.
  - x^T build: the x chunk DMA loads partition p <- rows {512c+4p+a}
    (2KB contiguous); transposing sub-tile a gives x^T columns l=4p+a at
    position 128a+32m+pp. The xt evacuation copy reorders to l-contiguous
    tile-major (dst-packed 4D AP; src strided breaks the DVE 2x mode but
    it's still ~0.7us vs v3's 1.3us un-interleave) so every matmul
    stationary AP stays 1-free-dim (a BIR verifier requirement) and the
    causal mask stays the plain triangular constant. The y store pays
    instead: partition p holds rows {512J+128t+p}, 4x512B runs.
  - causal mask add is a REGULAR f16 matmul (ident_h stationary, permuted
    mask as moving operand, accumulate into the diag strip) instead of
    v3's fp32 transpose-matmul: half the PE cost, same PE-only dep chain.
  - sums col-tiled: segments round-robin over 4 distinct 32-column PE
    array groups (ones[128,32] stationary, tile_position=(0,32g)) which
    run CONCURRENTLY -> ~4x less PE time than v3's full-width DR sums.
    Bands accumulate partials; epilogue contracts 4 bands with a 0/1
    selector column (the sel matmuls also transpose sums to per-partition
    layout for the reciprocal).
  - epilogue O^T->O transposes moved off the PE onto the DMA XBAR
    (dma_start_transpose, 4x [128,128] f16 per block on idle DMA HW).
  - late epilogue PSUM (sel-matmul output) borrows the dead tail of the
    block's diag2 pair slot (its strip is only 384 wide); late is deferred
    exactly ONE pair into the next block so the borrow's slot isn't yet
    reused and the smr copy has drained.
"""
import sys

if '/opt/trn_rl_repo' not in sys.path:
    sys.path.insert(0, '/opt/trn_rl_repo')

import numpy as np

B, L, D, H = 16, 4096, 128, 128
NCORES = 8
BPC = B // NCORES          # batches per core
QB = 512                   # q block width
NQB = L // QB              # 8 q blocks
KT = 128                   # k tile width
NKT = L // KT              # 32 k tiles
CHUNK = 512                # phase-A l-chunk
NCHUNK = L // CHUNK        # 8
SCALE = float(1.0 / np.sqrt(H))
NEG = -60000.0             # fp16-representable; SCALE*NEG << -80 so exp==0

_cache = {}


def _build():
    import concourse.mybir as mybir
    import concourse.tile as tile
    from concourse import bacc

    f32 = mybir.dt.float32
    f16 = mybir.dt.float16
    f8 = mybir.dt.float8e4
    DR = mybir.MatmulPerfMode.DoubleRow
    EXP = mybir.ActivationFunctionType.Exp

    nc = bacc.Bacc("TRN2", target_bir_lowering=False, debug=False,
                   num_devices=NCORES)
    x_ap = nc.dram_tensor("x16", [BPC, L, D], f16, kind="ExternalInput").ap()
    wq_ap = nc.dram_tensor("Wq16", [D, H], f16, kind="ExternalInput").ap()
    wk_ap = nc.dram_tensor("Wk16", [D, H], f16, kind="ExternalInput").ap()
    wv_ap = nc.dram_tensor("Wv16", [D, H], f16, kind="ExternalInput").ap()
    id_ap = nc.dram_tensor("ident16", [128, 128], f16, kind="ExternalInput").ap()
    mk_ap = nc.dram_tensor("mask16", [128, 128], f16, kind="ExternalInput").ap()
    sel_ap = nc.dram_tensor("sel16", [128, 1], f16, kind="ExternalInput").ap()
    y_ap = nc.dram_tensor("y", [BPC, L, H], f32, kind="ExternalOutput").ap()

    with tile.TileContext(nc) as tc:
        with (
            tc.tile_pool(name="const", bufs=1) as constp,
            tc.tile_pool(name="xchunk", bufs=4) as xchp,
            tc.tile_pool(name="xt", bufs=3) as xtp,
            tc.tile_pool(name="qkv", bufs=BPC) as qkvp,
            tc.tile_pool(name="pt", bufs=8) as ptp,
            tc.tile_pool(name="otsb", bufs=2) as otsbp,
            tc.tile_pool(name="smsb", bufs=2) as smsbp,
            tc.tile_pool(name="ysb", bufs=3) as yp,
            tc.tile_pool(name="ps_mm", bufs=2, space="PSUM") as ps_mm,
            tc.tile_pool(name="ps_a", bufs=2, space="PSUM") as ps_a,
            tc.tile_pool(name="ps_ot", bufs=1, space="PSUM") as ps_ot,
            tc.tile_pool(name="ps_sums", bufs=1, space="PSUM") as ps_sums,
        ):
            # ---- constants (all f16 direct from host; ACT stays exp-only) ----
            ident_h = constp.tile([128, 128], f16, tag="ident_h")
            nc.scalar.dma_start(ident_h[:], id_ap[:])
            mask_h = constp.tile([128, 128], f16, tag="mask_h")
            nc.scalar.dma_start(mask_h[:], mk_ap[:])
            sel_h = constp.tile([128, 1], f16, tag="sel_h")
            nc.scalar.dma_start(sel_h[:], sel_ap[:])
            w_h = {}
            for name, ap in (("q", wq_ap), ("k", wk_ap), ("v", wv_ap)):
                wh = constp.tile([128, 128], f16, tag=f"w{name}h")
                nc.scalar.dma_start(wh[:], ap[:])
                w_h[name] = wh
            ones_h = constp.tile([128, 32], f16, tag="ones_h")
            nc.gpsimd.memset(ones_h[:], 1.0)
            ones_8 = constp.tile([128, 32], f8, tag="ones_8")
            nc.gpsimd.memset(ones_8[:], 1.0)

            # ---- per-batch tensors ----
            qT = {}
            kT = {}
            Vn = {}
            Vh = {}
            xvs = {}
            yvs = {}
            for b in range(BPC):
                qT[b] = qkvp.tile([128, L], f16, tag="qT", name=f"qT{b}")
                kT[b] = qkvp.tile([128, L], f16, tag="kT", name=f"kT{b}")
                Vn[b] = qkvp.tile([128, L], f8, tag="V", name=f"V{b}")
                Vh[b] = qkvp.tile([128, L], f16, tag="Vh", name=f"Vh{b}")
                xvs[b] = x_ap[b].rearrange("(c p a) d -> c p (a d)", p=128, a=4)
                yvs[b] = y_ap[b].rearrange("(g t p) h -> g p t h", t=4, p=128)

            # ---- phase A granules (per chunk c of batch b) ----
            # x chunk DMA: partition p gets rows {512c+4p+a}, 2KB contiguous.
            # PSUM: 4 rotating 1-bank allocs from ps_a (tp, q, k, V); each
            # granule's PE part only ever waits on one of our own DVE
            # evacuations, never on a pair's exp.
            def phase_a1(b, c):
                xch = xchp.tile([128, 512], f16, tag="xch")
                nc.sync.dma_start(xch[:], xvs[b][c])
                tp_f32 = ps_a.tile([128, 512], f32, tag="a", name="tp")
                tp = tp_f32.bitcast(f16)[:, 0:512]
                for n in range(4):
                    nc.tensor.transpose(
                        tp[:, 128 * n:128 * (n + 1)],
                        xch[:, 128 * n:128 * (n + 1)], ident_h[:])
                return tp

            def phase_a2(b, c, tp):
                # reorder to l-contiguous: tp col 128a+32m+pp holds
                # l=4(32m+pp)+a; dst col 128m+4pp+a. dst last dim packed,
                # src strided (1 elem/cycle on DVE, ~0.7us).
                xt = xtp.tile([128, CHUNK], f16, tag="xt")
                nc.vector.tensor_copy(
                    xt[:].rearrange("d (m p a) -> d m p a", p=32, a=4),
                    tp.rearrange("d (a m p) -> d m p a", a=4, m=4))
                return xt

            def phase_a3(b, c, xt):
                pq = ps_a.tile([128, 512], f32, tag="a", name="pq")
                nc.tensor.matmul(pq[:], w_h["q"][:], xt[:],
                                 start=True, stop=True)
                nc.vector.tensor_copy(
                    qT[b][:, CHUNK * c:CHUNK * (c + 1)], pq[:])

            def phase_a4(b, c, xt):
                pk = ps_a.tile([128, 512], f32, tag="a", name="pk")
                nc.tensor.matmul(pk[:], w_h["k"][:], xt[:],
                                 start=True, stop=True)
                nc.vector.tensor_copy(
                    kT[b][:, CHUNK * c:CHUNK * (c + 1)], pk[:])

            def phase_a5(b, c, xt):
                # V tiles: per k-tile m of the chunk, lhsT = the 128 xt
                # columns with l in that tile = 2D AP [a:4][p:32m..32m+32];
                # V-tile partition s then holds k = 128i + sigma(s), the
                # same order the S^T stationary loads use.
                pv = ps_a.tile([128, 512], f32, tag="a", name="pv")
                for m in range(4):
                    nc.tensor.matmul(pv[:, 128 * m:128 * (m + 1)],
                                     xt[:, 128 * m:128 * (m + 1)],
                                     w_h["v"][:], start=True, stop=True)
                nc.vector.tensor_copy(
                    Vh[b][:, CHUNK * c:CHUNK * (c + 1)], pv[:])
                nc.gpsimd.tensor_copy(
                    Vn[b][:, CHUNK * c:CHUNK * (c + 1)],
                    Vh[b][:, CHUNK * c:CHUNK * (c + 1)])

            # ---- phase B block: pairs + col-tiled sums, PV pipelined ----
            # The pair loop is software-pipelined by one: pair g's PV+sums
            # are emitted only after pair g+1's S^T+exp (across block
            # boundaries too), so the in-order PE queue never has a
            # PV-waiting-on-exp at its head blocking the next S^T.
            # dstate carries {"pv": closure, "late": closure} across blocks.
            def phase_b(b, J, feed, dstate):
                ot = ps_ot.tile([128, QB], f32, tag="ot")
                sm = ps_sums.tile([128, QB], f32, tag="sums")

                pairs = []
                for g in range(2 * J):
                    pairs.append((2 * g, 2 * g + 1))
                pairs.append((4 * J, 4 * J + 1))
                pairs.append((4 * J + 2, 4 * J + 3))

                # sums: each k-tile segment is one [K=128, M=32, N] matmul
                # with ones stationary on col group (seg%4); the 4 matmuls
                # of a flush hit distinct 32-col PE groups and execute
                # concurrently. Bands hold partials; epilogue combines.
                # sm is zeroed lazily at the first flush: the previous
                # block's sums epilogue (emitted inside the carried-over PV
                # closure) must precede it in emission order.
                blk = {"seg": 0, "npv": 0, "zeroed": False, "diag1": None}
                pending = []
                nseg_total = 4 * J + 4
                npv_total = 2 * J + 4

                def flush_sums():
                    if not blk["zeroed"]:
                        # start=False accumulation needs an explicit zero
                        # (start=True clears are per-written-row-range)
                        nc.vector.memset(sm[:], 0.0)
                        blk["zeroed"] = True
                    for kind, src, qoff in pending:
                        g = blk["seg"] % 4
                        last = blk["seg"] == nseg_total - 1
                        ones = ones_8 if kind == "f8" else ones_h
                        nc.tensor.matmul(
                            sm[32 * g:32 * (g + 1), qoff:], ones[:], src,
                            start=False, stop=last,
                            skip_group_check=True,
                            tile_position=(0, 32 * g))
                        blk["seg"] += 1
                    pending.clear()

                def make_late(diag1, otsb, smr):
                    def late():
                        # sel matmuls: combine the 4 col-group bands AND
                        # transpose sums to per-partition layout. stp
                        # borrows the dead tail of this block's diag1 pair
                        # slot (its strip is only 896 wide).
                        stp = diag1[:, 896:1024]
                        for a in range(4):
                            # independent 1-col groups: start=True each
                            nc.tensor.matmul(stp[:, a:a + 1],
                                             smr[:, 128 * a:128 * (a + 1)],
                                             sel_h[:],
                                             start=True, stop=True,
                                             skip_group_check=True)
                        rcp = smsbp.tile([128, 4], f32, tag="rcp")
                        nc.vector.reciprocal(rcp[:], stp[:, 0:4])
                        # O^T -> O: f16 PE transposes into a borrowed
                        # (phase-A pool) PSUM slot, then normalize on DVE.
                        # (DMA XBAR transposes were tried: 1.25us
                        # serialized sync-queue triggers gated the DVE
                        # queue and starved the PE.)
                        opt = ps_a.tile([128, 512], f32, tag="a", name="op")
                        op = opt.bitcast(f16)[:, 0:512]
                        ys = yp.tile([128, QB], f32, tag="y")
                        for a in range(4):
                            nc.tensor.transpose(
                                op[:, 128 * a:128 * (a + 1)],
                                otsb[:, 128 * a:128 * (a + 1)], ident_h[:])
                        for a in range(4):
                            nc.vector.tensor_scalar_mul(
                                ys[:, 128 * a:128 * (a + 1)],
                                op[:, 128 * a:128 * (a + 1)],
                                rcp[:, a:a + 1])
                        nc.sync.dma_start(
                            yvs[b][J],
                            ys[:].rearrange("p (t h) -> p t h", t=4))
                    return late

                def make_pv(pair, diag, entries, ptw, last_pair):
                    def pv():
                        if not diag:
                            i0 = pair[0]
                            nc.tensor.matmul(
                                ot[:, 0:QB],
                                Vn[b][:, KT * i0:KT * (i0 + 2)].rearrange(
                                    "p (o h) -> p o h", o=2),
                                ptw[:, 0:2 * QB].rearrange(
                                    "p (o q) -> p o q", o=2),
                                start=(blk["npv"] == 0),
                                stop=(blk["npv"] == npv_total - 1),
                                skip_group_check=True, perf_mode=DR)
                            blk["npv"] += 1
                            pending.append(("f8", ptw[:, 0:QB], 0))
                            pending.append(("f8", ptw[:, QB:2 * QB], 0))
                        else:
                            for i, qoff, off, N in entries:
                                nc.tensor.matmul(
                                    ot[:, qoff:],
                                    Vh[b][:, KT * i:KT * (i + 1)],
                                    ptw[:, off:off + N],
                                    start=(blk["npv"] == 0),
                                    stop=(blk["npv"] == npv_total - 1),
                                    skip_group_check=True)
                                blk["npv"] += 1
                                pending.append(
                                    ("f16", ptw[:, off:off + N], qoff))
                        if len(pending) >= 4:
                            flush_sums()
                        if last_pair:
                            # block done: evacuate ot and sm on the DVE,
                            # hand the PE epilogue to dstate["late"]
                            otsb = otsbp.tile([128, QB], f16, tag="otsb")
                            nc.vector.tensor_copy(otsb[:], ot[:])
                            smr = smsbp.tile([128, QB], f16, tag="smsb")
                            nc.vector.tensor_copy(smr[:], sm[:])
                            dstate["late"] = make_late(
                                blk["diag1"], otsb, smr)
                    return pv

                for pi, pair in enumerate(pairs):
                    stw = ps_mm.tile([128, 2 * QB], f32, tag="mm")
                    diag = pair[0] >= 4 * J
                    entries = []
                    cur = 0
                    for i in pair:
                        qoff = max(0, 128 * (i - 4 * J))
                        N = QB - qoff
                        nc.tensor.matmul(
                            stw[:, cur:cur + N],
                            kT[b][:, KT * i:KT * (i + 1)],
                            qT[b][:, QB * J + qoff:QB * (J + 1)],
                            start=True, stop=not diag,
                            skip_group_check=True)
                        if diag:
                            # causal mask on the triangle block: regular
                            # f16 matmul accumulate (ident stationary,
                            # mask moving) -- half the cost of v3's fp32
                            # transpose-matmul, same PE-only dep chain.
                            nc.tensor.matmul(
                                stw[:, cur:cur + 128],
                                ident_h[:], mask_h[:],
                                start=False, stop=True,
                                skip_group_check=True)
                        entries.append((i, qoff, cur, N))
                        cur += N
                    ptw = ptp.tile([128, 2 * QB], f16 if diag else f8,
                                   tag="pt" if diag else "pt8")
                    nc.scalar.activation(ptw[:, :cur], stw[:, :cur], EXP,
                                         scale=SCALE)
                    if pair[0] == 4 * J:
                        blk["diag1"] = stw
                    if dstate["pv"] is not None:
                        dstate["pv"]()
                        dstate["pv"] = None
                    if pi == 1 and dstate["late"] is not None:
                        dstate["late"]()
                        dstate["late"] = None
                    if feed is not None:
                        feed()
                    dstate["pv"] = make_pv(pair, diag, entries, ptw,
                                           pi == len(pairs) - 1)

            # ---- interleaved emission schedule ----
            gran = []
            for b in range(BPC):
                for c in range(NCHUNK):
                    gran.append((b, c))
            gstate = {"idx": 0, "sub": 0, "tp": None, "xt": None,
                      "pairs": 0}
            NSUB = 5

            def emit_sub():
                b, c = gran[gstate["idx"]]
                s = gstate["sub"]
                if s == 0:
                    gstate["tp"] = phase_a1(b, c)
                elif s == 1:
                    gstate["xt"] = phase_a2(b, c, gstate["tp"])
                elif s == 2:
                    phase_a3(b, c, gstate["xt"])
                elif s == 3:
                    phase_a4(b, c, gstate["xt"])
                else:
                    phase_a5(b, c, gstate["xt"])
                gstate["sub"] += 1
                if gstate["sub"] == NSUB:
                    gstate["sub"] = 0
                    gstate["idx"] += 1

            def gpos():
                return gstate["idx"] * NSUB + gstate["sub"]

            NGRAN = len(gran) * NSUB
            NPAIRS = sum(2 * J + 2 for J in range(NQB)) * BPC

            def feed():
                gstate["pairs"] += 1
                target = min(NGRAN, 8 + gstate["pairs"] * NGRAN // NPAIRS)
                while gpos() < target:
                    emit_sub()

            border = [(0, J) for J in range(4)]
            for J in range(4):
                border += [(0, 4 + J), (1, J)]
            border += [(1, J) for J in range(4, NQB)]

            dstate = {"pv": None, "late": None}
            for b, J in border:
                while gpos() < ((b * NCHUNK + J) + 1) * NSUB:
                    emit_sub()
                phase_b(b, J, feed, dstate)
            if dstate["pv"] is not None:
                dstate["pv"]()
            if dstate["late"] is not None:
                dstate["late"]()
    nc.compile()
    return nc


def _host_consts():
    ident = np.eye(128, dtype=np.float16)
    # rows s (S^T k-row), cols j (q position within the triangle block):
    # keep iff q >= k <=> j >= s
    t = np.arange(128)
    mask = np.where(t[None, :] >= t[:, None], 0.0, NEG).astype(np.float16)
    sel = np.zeros((128, 1), dtype=np.float16)
    sel[0::32, 0] = 1.0
    return ident, mask, sel


def kernel(x, Wq, Wk, Wv):
    from concourse import bass_utils

    if "nc" not in _cache:
        _cache["nc"] = _build()
    nc = _cache["nc"]

    x16 = np.ascontiguousarray(x, dtype=np.float16)
    ident, mask, sel = _host_consts()
    in_maps = []
    for c in range(NCORES):
        in_maps.append({
            "x16": x16[BPC * c:BPC * (c + 1)],
            "Wq16": np.ascontiguousarray(Wq, dtype=np.float16),
            "Wk16": np.ascontiguousarray(Wk, dtype=np.float16),
            "Wv16": np.ascontiguousarray(Wv, dtype=np.float16),
            "ident16": ident,
            "mask16": mask,
            "sel16": sel,
        })
    res = bass_utils.run_bass_kernel_spmd(nc, in_maps,
                                          core_ids=list(range(NCORES)))
    _cache["last_results"] = res
    y = np.concatenate([res.results[c]["y"] for c in range(NCORES)], axis=0)
    return y
